# revision 85
# baseline (speedup 1.0000x reference)
"""Trainium2 Bass kernel: 3x3 conv (N=16, C_in=16, C_out=64, H=W=256, pad=1).

Strategy (8 NeuronCores, data-parallel over batch N -> 2 images/core):
  - All device I/O in fp16 (host converts): DMA floor ~21MB/core.
  - Per 64-row strip: slab [128 partitions = (kw d in {0,1}, row-slot s in
    {0..3}, ci)] holds 4-row groups at stride 2; partition (d,s,ci) at free
    (g, j) = xpad[ci, h0+2g+s-1, j+d].  One HBM load fills (d=0, s in {0,1});
    three DVE copies (4x perf mode) build the rest from it.
  - 2 matmul passes per psum tile (vs 3 in the kh-block scheme): pass1
    contracts all 128 partitions (kw0+kw1 taps), pass2 contracts the kw1
    block read at +1px (= kw2 taps).  M = 128 = (row-phase ph in {0,1}) x
    64 channels -> 131072 psum rows total = PE structural floor.
  - PSUM -> SBUF evac with fp32->fp16 convert split across Scalar/GpSimd;
    batched 64-partition stores (even/odd output rows).
"""

import sys

if "/opt/trn_rl_repo" not in sys.path:
    sys.path.insert(0, "/opt/trn_rl_repo")

import numpy as np

import concourse.bacc as bacc
import concourse.bass as bass
import concourse.mybir as mybir
import concourse.tile as tile
from concourse.bass_utils import run_bass_kernel_spmd

N_FULL, CI, CO, H, W_SP = 16, 16, 64, 256, 256
NCORES = 8
NB = N_FULL // NCORES          # images per core
HP, WP = H + 2, W_SP + 2       # padded image dims (258)
RSTRIP = 64                    # output rows per strip
NSS = H // RSTRIP              # strips per image (4)
G = RSTRIP // 2                # row-pairs per strip (32)
GH = G + 1                     # loaded groups (incl. halo group)
NT = G // 2                    # psum tiles per strip (16)
F32 = mybir.dt.float32
F16 = mybir.dt.float16

_CACHE = {}


def _build():
    nc = bacc.Bacc("TRN2", target_bir_lowering=False, debug=False)
    EB = 8                     # psum tiles per evac batch (32 output rows)
    NEB = NT // EB             # evac batches per strip (2)

    x_d = nc.dram_tensor("xp", [NB, CI, HP, WP], F16, kind="ExternalInput").ap()
    w_d = nc.dram_tensor("wts", [192, 128], F16, kind="ExternalInput").ap()
    # device-layout output: [n, strip, batch, (ph,co), (t', gi, j)];
    # host un-permutes to NCHW (out row = 64t + 32b + 4t' + 2gi + ph)
    o_d = nc.dram_tensor(
        "out", [NB, NSS, NEB, 128, EB * 512], F16, kind="ExternalOutput"
    ).ap()

    xe_n = CI * HP * WP        # x_pad element strides
    xe_c = HP * WP
    xe_h = WP

    with tile.TileContext(nc) as tc:
        with (
            tc.tile_pool(name="wp", bufs=1) as wpool,
            tc.tile_pool(name="slab", bufs=4) as slabpool,
            tc.tile_pool(name="evac", bufs=4) as evacpool,
            tc.tile_pool(name="ps", bufs=4, space="PSUM") as pspool,
        ):
            wsb = wpool.tile([128, 256], F16)
            # w1 = wsb[:, 0:128] (pass1, all 128 partitions)
            # w2 = wsb[64:128, 128:256] (pass2, kw1 block partitions)
            # (loads emitted in the prologue, after slab0's loads)

            def copies(sv, g0, g1, pool_c1=0):
                # c1:  kw0_s23[g] = kw0_s01[g+1]      (rows 2g+1, 2g+2)
                #      (optionally first `pool_c1` groups on GpSimd)
                # c23: kw1[g,j]   = kw0[g,j+1]        (64-partition copy)
                if pool_c1:
                    nc.gpsimd.tensor_copy(
                        sv[32:64, g0 : g0 + pool_c1, 0:WP],
                        sv[0:32, g0 + 1 : g0 + pool_c1 + 1, 0:WP],
                    )
                nc.vector.tensor_copy(
                    sv[32:64, g0 + pool_c1 : g1, 0:WP],
                    sv[0:32, g0 + pool_c1 + 1 : g1 + 1, 0:WP],
                )
                nc.vector.tensor_copy(
                    sv[64:128, g0:g1, 0 : WP - 1],
                    sv[0:64, g0:g1, 1:WP],
                )

            def load_slab(n, t, eng=None):
                # slab covers the whole strip: groups 0..G-1 (+ halo slot G)
                slab = slabpool.tile([128, GH * WP], F16, tag="slab")
                sf = slab[:]
                for s in range(2):
                    src = bass.AP(
                        x_d.tensor,
                        n * xe_n + (RSTRIP * t + s) * xe_h,
                        [[xe_c, CI], [2 * xe_h, GH], [1, WP]],
                    )
                    (eng or nc.sync).dma_start(sf[16 * s : 16 * s + 16, :], src)
                return slab

            def load_slab_extras(slab, n, t, eng):
                # fill s23 / kw1 blocks straight from HBM (no DVE copies);
                # only worthwhile while the DMA queue still has slack
                sf = slab[:]
                base = n * xe_n + RSTRIP * t * xe_h
                for s in (2, 3):     # kw0_s23: x-rows 2g+s-1, g in 0..G-1
                    src = bass.AP(
                        x_d.tensor,
                        base + s * xe_h,
                        [[xe_c, CI], [2 * xe_h, G], [1, WP]],
                    )
                    eng.dma_start(
                        sf[16 * s : 16 * s + 16, 0 : G * WP], src
                    )
                sv = sf.rearrange("p (g j) -> p g j", j=WP)
                for s in range(4):   # kw1: same rows, +1 px, 257 cols
                    src = bass.AP(
                        x_d.tensor,
                        base + s * xe_h + 1,
                        [[xe_c, CI], [2 * xe_h, G], [1, WP - 1]],
                    )
                    eng.dma_start(
                        sv[64 + 16 * s : 80 + 16 * s, 0:G, 0 : WP - 1], src
                    )

            def compute(n, t, slab, all_scalar=False, fine_stores=False,
                        dve_evacs=(3, 7), ebs=range(4)):
                sv = slab[:].rearrange("p (g j) -> p g j", j=WP)
                for eb in ebs:
                    evac = evacpool.tile([128, 2048], F16, tag="evac")
                    for pth in range(2):
                        pt = 2 * eb + pth
                        ps = pspool.tile([128, 1024], F32, tag="ps")
                        for q in range(2):
                            g0 = 2 * (2 * pt + q)
                            nc.tensor.matmul(
                                ps[:, q * 512 : (q + 1) * 512],
                                wsb[0:128, 0:128],
                                sv[0:128, g0 : g0 + 2, 0:256],
                                start=True,
                                stop=False,
                            )
                            nc.tensor.matmul(
                                ps[:, q * 512 : (q + 1) * 512],
                                wsb[64:128, 128:256],
                                sv[64:128, g0 : g0 + 2, 1:257],
                                start=False,
                                stop=True,
                            )
                        if fine_stores:
                            dve = pt in dve_evacs
                            op = (nc.vector.tensor_copy if dve
                                  else nc.scalar.copy)
                            op(evac[:, pth * 1024 : (pth + 1) * 1024], ps[:])
                            dst = bass.AP(
                                o_d.tensor,
                                (((n * NSS + t) * 4 + eb) * 128 * 2048
                                 + pth * 1024),
                                [[2048, 128], [1, 1024]],
                            )
                            nc.sync.dma_start(
                                dst, evac[:, pth * 1024 : (pth + 1) * 1024]
                            )
                        else:
                            dve = (pt in dve_evacs) and not all_scalar
                            op = (nc.vector.tensor_copy if dve
                                  else nc.scalar.copy)
                            op(evac[:, pth * 1024 : (pth + 1) * 1024], ps[:])
                    if not fine_stores:
                        dst = bass.AP(
                            o_d.tensor,
                            ((n * NSS + t) * 4 + eb) * 128 * 2048,
                            [[2048, 128], [1, 2048]],
                        )
                        nc.sync.dma_start(dst, evac[:])

            # --- software pipeline ---------------------------------------
            # Loads run 2 steps ahead (SP queue, ahead of stores); copies run
            # 1 step ahead and are emitted BEFORE each step's evacs so the
            # DVE never idles on matmul waits while copies are ready.
            steps = [(n, t) for n in range(NB) for t in range(NSS)]

            # prologue: slab 0 in chunks for a fast start (dep tracking is
            # region-precise, so early matmuls run on partial slabs);
            # high_priority pins these ahead of later copies in the scheduler
            slab0 = slabpool.tile([128, GH * WP], F16, tag="slab")
            sv0 = slab0[:].rearrange("p (g j) -> p g j", j=WP)
            CHUNKS = (0, 8, 16, 24, 32)   # pair-group boundaries
            with tc.high_priority():
                # disjoint load ranges (no WAR chains between chunks)
                for c in range(4):
                    ga = 0 if c == 0 else CHUNKS[c] + 1
                    gb = CHUNKS[c + 1] + 1
                    leng = nc.sync if c < 2 else nc.gpsimd
                    for s in range(2):
                        src = bass.AP(
                            x_d.tensor,
                            (2 * ga + s) * xe_h,
                            [[xe_c, CI], [2 * xe_h, gb - ga], [1, WP]],
                        )
                        leng.dma_start(
                            slab0[:][16 * s : 16 * s + 16,
                                     ga * WP : gb * WP],
                            src,
                        )
                    if c == 0:
                        nc.sync.dma_start(wsb[0:128, 0:128], w_d[0:128, :])
                        nc.sync.dma_start(wsb[64:128, 128:256],
                                          w_d[128:192, :])
                for c in range(4):
                    copies(sv0, CHUNKS[c], CHUNKS[c + 1])
            slabs = {steps[0]: slab0}
            slabs[steps[2]] = load_slab(*steps[2], eng=nc.sync)
            slabs[steps[1]] = load_slab(*steps[1], eng=nc.gpsimd)
            load_slab_extras(slabs[steps[1]], *steps[1], nc.sync)
            for i, (n, t) in enumerate(steps):
                if i + 3 < len(steps):
                    slabs[steps[i + 3]] = load_slab(*steps[i + 3])
                nxt = (slabs[steps[i + 2]][:].rearrange(
                           "p (g j) -> p g j", j=WP)
                       if i + 2 < len(steps) else None)
                last = i == len(steps) - 1
                slab = slabs.pop((n, t))
                kw = dict(all_scalar=(i == 0), fine_stores=last,
                          dve_evacs=(1, 3, 5, 7) if last else (2, 5))
                if nxt is not None:
                    # DVE stream: c1(i+2), evac(i,3), c23(i+2), evac(i,6)
                    nc.vector.tensor_copy(
                        nxt[32:64, 0:G, 0:WP],
                        nxt[0:32, 1 : G + 1, 0:WP],
                    )
                    compute(n, t, slab, ebs=range(2), **kw)
                    nc.vector.tensor_copy(
                        nxt[64:128, 0:G, 0 : WP - 1],
                        nxt[0:64, 0:G, 1:WP],
                    )
                    compute(n, t, slab, ebs=range(2, 4), **kw)
                else:
                    compute(n, t, slab, **kw)

    nc.compile()
    return nc


def _prep_weights(W: np.ndarray) -> np.ndarray:
    # lhsT layouts, stored stacked as [192, 128] then transposed on load:
    #   w1[(d,s,ci), (ph,co)] = W[co, ci, s-ph, d]    (rows 0..127)
    #   w2[(s,ci),   (ph,co)] = W[co, ci, s-ph, 2]    (rows 128..191)
    w = np.zeros((192, 128), dtype=np.float32)
    for s in range(4):
        for ph in range(2):
            kh = s - ph
            if not (0 <= kh <= 2):
                continue
            blk = W[:, :, kh, :]  # [co, ci, kw]
            for d in range(2):
                w[d * 64 + s * 16 : d * 64 + (s + 1) * 16,
                  ph * 64 : (ph + 1) * 64] = blk[:, :, d].T
            w[128 + s * 16 : 128 + (s + 1) * 16,
              ph * 64 : (ph + 1) * 64] = blk[:, :, 2].T
    return w.astype(np.float16)


def _prep_inputs(x: np.ndarray, W: np.ndarray) -> list[dict]:
    wts = _prep_weights(np.asarray(W, dtype=np.float32))
    xs = np.asarray(x, dtype=np.float32).reshape(NCORES, NB, CI, H, W_SP)
    in_maps = []
    for i in range(NCORES):
        xp = np.zeros((NB, CI, HP, WP), dtype=np.float16)
        xp[:, :, 1 : H + 1, 1 : W_SP + 1] = xs[i]
        in_maps.append({"xp": xp, "wts": wts})
    return in_maps


def kernel(x: np.ndarray, W: np.ndarray) -> np.ndarray:
    assert x.shape == (N_FULL, CI, H, W_SP) and W.shape == (CO, CI, 3, 3)
    # BASS_TRACE without the axon NTFF hook module would crash the run path;
    # disable tracing only when the hook is genuinely unavailable.
    try:
        import antenv.axon_hooks  # noqa: F401
    except Exception:
        import os

        os.environ.setdefault("BASS_NEVER_TRACE", "1")
    if "nc" not in _CACHE:
        _CACHE["nc"] = _build()
    nc = _CACHE["nc"]

    in_maps = _prep_inputs(x, W)
    res = run_bass_kernel_spmd(nc, in_maps, list(range(NCORES)))
    parts = []
    for i in range(NCORES):
        dev = np.asarray(res.results[i]["out"], dtype=np.float32)
        # [n, t, eb, ph, co, h, gi, j] -> [n, co, (t eb h gi ph), j]
        # (out row = 64t + 16eb + 4h + 2gi + ph)
        dev = dev.reshape(NB, NSS, 4, 2, CO, 4, 2, 256)
        dev = dev.transpose(0, 4, 1, 2, 5, 6, 3, 7).reshape(NB, CO, H, W_SP)
        parts.append(dev)
    return np.concatenate(parts, axis=0)


# revision 103
# speedup vs baseline: 1.0354x; 1.0354x over previous
"""Trainium2 Bass kernel: 3x3 conv (N=16, C_in=16, C_out=64, H=W=256, pad=1).

Strategy (8 NeuronCores, data-parallel over batch N -> 2 images/core):
  - All device I/O in fp16 (host converts): ~21MB/core = 58.7us DMA floor
    at the cost model's 360GB/s; rel err vs fp32 reference ~4.5e-4.
  - Per 64-row strip: slab [128 partitions = (kw d in {0,1}, row-slot s in
    {0..3}, ci)] holds 4-row groups at stride 2; partition (d,s,ci) at free
    (g, j) = xpad[ci, h0+2g+s-1, j+d].  Two HBM loads fill (d=0, s in
    {0,1}); two DVE copies (4x perf mode) build s23 and the kw1 block.
  - 2 matmul passes per psum region (vs 3 in the kh-block scheme): pass1
    contracts all 128 partitions (kw0+kw1 taps), pass2 contracts the kw1
    block read at +1px (= kw2 taps).  M = 128 = (row-phase ph in {0,1}) x
    64 channels -> 131072 psum rows total = the PE structural floor (every
    output needs ceil(144/128) = 2 column visits).
  - PSUM -> SBUF evac (fp32->fp16) split 6:2 scalar:vector per strip;
    contiguous device-layout stores (host un-permutes rows).
  - Pipeline: loads 3 steps ahead (SP queue, ahead of stores), copies 2
    ahead emitted around the compute halves so the DVE stream is
    [c1, evac, c23, evac]; first strip loads/copies in disjoint quarters
    (region-precise deps let matmuls start on a partial slab); slab1's
    dup blocks direct-loaded from HBM during the early DMA-idle window.
  Steady-state periods run at the DMA roofline (7.34us per 64-row strip).
"""

import sys

if "/opt/trn_rl_repo" not in sys.path:
    sys.path.insert(0, "/opt/trn_rl_repo")

import numpy as np

import concourse.bacc as bacc
import concourse.bass as bass
import concourse.mybir as mybir
import concourse.tile as tile
from concourse.bass_utils import run_bass_kernel_spmd

N_FULL, CI, CO, H, W_SP = 16, 16, 64, 256, 256
NCORES = 8
NB = N_FULL // NCORES          # images per core
HP, WP = H + 2, W_SP + 2       # padded image dims (258)
RSTRIP = 64                    # output rows per strip
NSS = H // RSTRIP              # strips per image (4)
G = RSTRIP // 2                # row-pairs per strip (32)
GH = G + 1                     # loaded groups (incl. halo group)
NT = G // 2                    # psum tiles per strip (16)
F32 = mybir.dt.float32
F16 = mybir.dt.float16

_CACHE = {}


def _build():
    nc = bacc.Bacc("TRN2", target_bir_lowering=False, debug=False)
    EB = 8                     # psum tiles per evac batch (32 output rows)
    NEB = NT // EB             # evac batches per strip (2)

    x_d = nc.dram_tensor("xp", [NB, CI, HP, WP], F16, kind="ExternalInput").ap()
    w_d = nc.dram_tensor("wts", [128, 256], F16, kind="ExternalInput").ap()
    # device-layout output: [n, strip, batch, (ph,co), (t', gi, j)];
    # host un-permutes to NCHW (out row = 64t + 32b + 4t' + 2gi + ph)
    o_d = nc.dram_tensor(
        "out", [NB, NSS, NEB, 128, EB * 512], F16, kind="ExternalOutput"
    ).ap()

    xe_n = CI * HP * WP        # x_pad element strides
    xe_c = HP * WP
    xe_h = WP

    with tile.TileContext(nc) as tc:
        with (
            tc.tile_pool(name="wp", bufs=1) as wpool,
            tc.tile_pool(name="slab", bufs=4) as slabpool,
            tc.tile_pool(name="evac", bufs=4) as evacpool,
            tc.tile_pool(name="ps", bufs=4, space="PSUM") as pspool,
        ):
            wsb = wpool.tile([128, 256], F16)
            # w1 = wsb[:, 0:128] (pass1, all 128 partitions)
            # w2 = wsb[64:128, 128:256] (pass2, kw1 block partitions)
            # (loads emitted in the prologue, after slab0's loads)

            def copies(sv, g0, g1, pool_c1=0):
                # c1:  kw0_s23[g] = kw0_s01[g+1]      (rows 2g+1, 2g+2)
                #      (optionally first `pool_c1` groups on GpSimd)
                # c23: kw1[g,j]   = kw0[g,j+1]        (64-partition copy)
                if pool_c1:
                    nc.gpsimd.tensor_copy(
                        sv[32:64, g0 : g0 + pool_c1, 0:WP],
                        sv[0:32, g0 + 1 : g0 + pool_c1 + 1, 0:WP],
                    )
                nc.vector.tensor_copy(
                    sv[32:64, g0 + pool_c1 : g1, 0:WP],
                    sv[0:32, g0 + pool_c1 + 1 : g1 + 1, 0:WP],
                )
                nc.vector.tensor_copy(
                    sv[64:128, g0:g1, 0 : WP - 1],
                    sv[0:64, g0:g1, 1:WP],
                )

            def load_slab(n, t, eng=None):
                # slab covers the whole strip: groups 0..G-1 (+ halo slot G)
                slab = slabpool.tile([128, GH * WP], F16, tag="slab")
                sf = slab[:]
                for s in range(2):
                    src = bass.AP(
                        x_d.tensor,
                        n * xe_n + (RSTRIP * t + s) * xe_h,
                        [[xe_c, CI], [2 * xe_h, GH], [1, WP]],
                    )
                    (eng or nc.sync).dma_start(sf[16 * s : 16 * s + 16, :], src)
                return slab

            def load_slab_extras(slab, n, t, eng, eng_s23=None):
                # fill s23 / kw1 blocks straight from HBM (no DVE copies);
                # only worthwhile while the DMA queue still has slack
                sf = slab[:]
                base = n * xe_n + RSTRIP * t * xe_h
                for s in (2, 3):     # kw0_s23: x-rows 2g+s-1, g in 0..G-1
                    src = bass.AP(
                        x_d.tensor,
                        base + s * xe_h,
                        [[xe_c, CI], [2 * xe_h, G], [1, WP]],
                    )
                    (eng_s23 or eng).dma_start(
                        sf[16 * s : 16 * s + 16, 0 : G * WP], src
                    )
                sv = sf.rearrange("p (g j) -> p g j", j=WP)
                for s in range(4):   # kw1: same rows, +1 px, 257 cols
                    src = bass.AP(
                        x_d.tensor,
                        base + s * xe_h + 1,
                        [[xe_c, CI], [2 * xe_h, G], [1, WP - 1]],
                    )
                    eng.dma_start(
                        sv[64 + 16 * s : 80 + 16 * s, 0:G, 0 : WP - 1], src
                    )

            def compute(n, t, slab, all_scalar=False, fine_stores=False,
                        dve_evacs=(3, 7), ebs=range(4)):
                sv = slab[:].rearrange("p (g j) -> p g j", j=WP)
                for eb in ebs:
                    evac = evacpool.tile([128, 2048], F16, tag="evac")
                    for pth in range(2):
                        pt = 2 * eb + pth
                        ps = pspool.tile([128, 1024], F32, tag="ps")
                        for q in range(2):
                            g0 = 2 * (2 * pt + q)
                            nc.tensor.matmul(
                                ps[:, q * 512 : (q + 1) * 512],
                                wsb[0:128, 0:128],
                                sv[0:128, g0 : g0 + 2, 0:256],
                                start=True,
                                stop=False,
                            )
                            nc.tensor.matmul(
                                ps[:, q * 512 : (q + 1) * 512],
                                wsb[64:128, 128:256],
                                sv[64:128, g0 : g0 + 2, 1:257],
                                start=False,
                                stop=True,
                            )
                        if fine_stores:
                            dve = pt in dve_evacs
                            op = (nc.vector.tensor_copy if dve
                                  else nc.scalar.copy)
                            op(evac[:, pth * 1024 : (pth + 1) * 1024], ps[:])
                            dst = bass.AP(
                                o_d.tensor,
                                (((n * NSS + t) * 4 + eb) * 128 * 2048
                                 + pth * 1024),
                                [[2048, 128], [1, 1024]],
                            )
                            nc.sync.dma_start(
                                dst, evac[:, pth * 1024 : (pth + 1) * 1024]
                            )
                        else:
                            dve = (pt in dve_evacs) and not all_scalar
                            op = (nc.vector.tensor_copy if dve
                                  else nc.scalar.copy)
                            op(evac[:, pth * 1024 : (pth + 1) * 1024], ps[:])
                    if not fine_stores:
                        dst = bass.AP(
                            o_d.tensor,
                            ((n * NSS + t) * 4 + eb) * 128 * 2048,
                            [[2048, 128], [1, 2048]],
                        )
                        nc.sync.dma_start(dst, evac[:])

            # --- software pipeline ---------------------------------------
            # Loads run 2 steps ahead (SP queue, ahead of stores); copies run
            # 1 step ahead and are emitted BEFORE each step's evacs so the
            # DVE never idles on matmul waits while copies are ready.
            steps = [(n, t) for n in range(NB) for t in range(NSS)]

            # prologue: slab 0 in chunks for a fast start (dep tracking is
            # region-precise, so early matmuls run on partial slabs);
            # high_priority pins these ahead of later copies in the scheduler
            slab0 = slabpool.tile([128, GH * WP], F16, tag="slab")
            sv0 = slab0[:].rearrange("p (g j) -> p g j", j=WP)
            CHUNKS = (0, 8, 16, 24, 32)   # pair-group boundaries
            with tc.high_priority():
                # disjoint load ranges (no WAR chains between chunks)
                for c in range(4):
                    ga = 0 if c == 0 else CHUNKS[c] + 1
                    gb = CHUNKS[c + 1] + 1
                    for s in range(2):
                        leng = (nc.sync if (c < 2 and (c > 0 or s == 0))
                                else nc.gpsimd)
                        src = bass.AP(
                            x_d.tensor,
                            (2 * ga + s) * xe_h,
                            [[xe_c, CI], [2 * xe_h, gb - ga], [1, WP]],
                        )
                        leng.dma_start(
                            slab0[:][16 * s : 16 * s + 16,
                                     ga * WP : gb * WP],
                            src,
                        )
                    if c == 0:
                        nc.sync.dma_start(wsb[:], w_d[0:128, :])
                for c in range(4):
                    copies(sv0, CHUNKS[c], CHUNKS[c + 1])
            slabs = {steps[0]: slab0}
            slabs[steps[2]] = load_slab(*steps[2], eng=nc.sync)
            slabs[steps[1]] = load_slab(*steps[1], eng=nc.gpsimd)
            load_slab_extras(slabs[steps[1]], *steps[1], nc.sync,
                             eng_s23=nc.gpsimd)
            for i, (n, t) in enumerate(steps):
                if i + 3 < len(steps):
                    slabs[steps[i + 3]] = load_slab(*steps[i + 3])
                nxt = (slabs[steps[i + 2]][:].rearrange(
                           "p (g j) -> p g j", j=WP)
                       if i + 2 < len(steps) else None)
                last = i == len(steps) - 1
                slab = slabs.pop((n, t))
                kw = dict(all_scalar=(i == 0), fine_stores=last,
                          dve_evacs=(1, 3, 5, 7) if last else (2, 5))
                if nxt is not None:
                    # DVE stream: c1(i+2), evac(i,3), c23(i+2), evac(i,6)
                    nc.vector.tensor_copy(
                        nxt[32:64, 0:G, 0:WP],
                        nxt[0:32, 1 : G + 1, 0:WP],
                    )
                    compute(n, t, slab, ebs=range(2), **kw)
                    nc.vector.tensor_copy(
                        nxt[64:128, 0:G, 0 : WP - 1],
                        nxt[0:64, 0:G, 1:WP],
                    )
                    compute(n, t, slab, ebs=range(2, 4), **kw)
                else:
                    compute(n, t, slab, **kw)

    nc.compile()
    return nc


def _prep_weights(W: np.ndarray) -> np.ndarray:
    # lhsT layouts, stored stacked as [192, 128] then transposed on load:
    #   w1[(d,s,ci), (ph,co)] = W[co, ci, s-ph, d]    (rows 0..127)
    #   w2[(s,ci),   (ph,co)] = W[co, ci, s-ph, 2]    (rows 128..191)
    w = np.zeros((128, 256), dtype=np.float32)
    for s in range(4):
        for ph in range(2):
            kh = s - ph
            if not (0 <= kh <= 2):
                continue
            blk = W[:, :, kh, :]  # [co, ci, kw]
            for d in range(2):
                w[d * 64 + s * 16 : d * 64 + (s + 1) * 16,
                  ph * 64 : (ph + 1) * 64] = blk[:, :, d].T
            w[64 + s * 16 : 64 + (s + 1) * 16,
              128 + ph * 64 : 128 + (ph + 1) * 64] = blk[:, :, 2].T
    return w.astype(np.float16)


def _prep_inputs(x: np.ndarray, W: np.ndarray) -> list[dict]:
    wts = _prep_weights(np.asarray(W, dtype=np.float32))
    xs = np.asarray(x, dtype=np.float32).reshape(NCORES, NB, CI, H, W_SP)
    in_maps = []
    for i in range(NCORES):
        xp = np.zeros((NB, CI, HP, WP), dtype=np.float16)
        xp[:, :, 1 : H + 1, 1 : W_SP + 1] = xs[i]
        in_maps.append({"xp": xp, "wts": wts})
    return in_maps


def kernel(x: np.ndarray, W: np.ndarray) -> np.ndarray:
    assert x.shape == (N_FULL, CI, H, W_SP) and W.shape == (CO, CI, 3, 3)
    # BASS_TRACE without the axon NTFF hook module would crash the run path;
    # disable tracing only when the hook is genuinely unavailable.
    try:
        import antenv.axon_hooks  # noqa: F401
    except Exception:
        import os

        os.environ.setdefault("BASS_NEVER_TRACE", "1")
    if "nc" not in _CACHE:
        _CACHE["nc"] = _build()
    nc = _CACHE["nc"]

    in_maps = _prep_inputs(x, W)
    res = run_bass_kernel_spmd(nc, in_maps, list(range(NCORES)))
    parts = []
    for i in range(NCORES):
        dev = np.asarray(res.results[i]["out"], dtype=np.float32)
        # [n, t, eb, ph, co, h, gi, j] -> [n, co, (t eb h gi ph), j]
        # (out row = 64t + 16eb + 4h + 2gi + ph)
        dev = dev.reshape(NB, NSS, 4, 2, CO, 4, 2, 256)
        dev = dev.transpose(0, 4, 1, 2, 5, 6, 3, 7).reshape(NB, CO, H, W_SP)
        parts.append(dev)
    return np.concatenate(parts, axis=0)


# revision 110
# speedup vs baseline: 1.0383x; 1.0028x over previous
"""Trainium2 Bass kernel: 3x3 conv (N=16, C_in=16, C_out=64, H=W=256, pad=1).

Strategy (8 NeuronCores, data-parallel over batch N -> 2 images/core):
  - All device I/O in fp16 (host converts): ~21MB/core = 58.7us DMA floor
    at the cost model's 360GB/s; rel err vs fp32 reference ~4.5e-4.
  - Per 64-row strip: slab [128 partitions = (kw d in {0,1}, row-slot s in
    {0..3}, ci)] holds 4-row groups at stride 2; partition (d,s,ci) at free
    (g, j) = xpad[ci, h0+2g+s-1, j+d].  Two HBM loads fill (d=0, s in
    {0,1}); two DVE copies (4x perf mode) build s23 and the kw1 block.
  - 2 matmul passes per psum region (vs 3 in the kh-block scheme): pass1
    contracts all 128 partitions (kw0+kw1 taps), pass2 contracts the kw1
    block read at +1px (= kw2 taps).  M = 128 = (row-phase ph in {0,1}) x
    64 channels -> 131072 psum rows total = the PE structural floor (every
    output needs ceil(144/128) = 2 column visits).
  - PSUM -> SBUF evac (fp32->fp16) split 6:2 scalar:vector per strip;
    contiguous device-layout stores (host un-permutes rows).
  - Pipeline: loads 3 steps ahead (SP queue, ahead of stores), copies 2
    ahead emitted around the compute halves so the DVE stream is
    [c1, evac, c23, evac]; first strip loads/copies in disjoint quarters
    (region-precise deps let matmuls start on a partial slab); slab1's
    dup blocks direct-loaded from HBM during the early DMA-idle window.
  Steady-state periods run at the DMA roofline (7.34us per 64-row strip).
"""

import sys

if "/opt/trn_rl_repo" not in sys.path:
    sys.path.insert(0, "/opt/trn_rl_repo")

import numpy as np

import concourse.bacc as bacc
import concourse.bass as bass
import concourse.mybir as mybir
import concourse.tile as tile
from concourse.bass_utils import run_bass_kernel_spmd

N_FULL, CI, CO, H, W_SP = 16, 16, 64, 256, 256
NCORES = 8
NB = N_FULL // NCORES          # images per core
HP, WP = H + 2, W_SP + 2       # padded image dims (258)
RSTRIP = 64                    # output rows per strip
NSS = H // RSTRIP              # strips per image (4)
G = RSTRIP // 2                # row-pairs per strip (32)
GH = G + 1                     # loaded groups (incl. halo group)
NT = G // 2                    # psum tiles per strip (16)
F32 = mybir.dt.float32
F16 = mybir.dt.float16

_CACHE = {}


def _build():
    nc = bacc.Bacc("TRN2", target_bir_lowering=False, debug=False)
    EB = 8                     # psum tiles per evac batch (32 output rows)
    NEB = NT // EB             # evac batches per strip (2)

    x_d = nc.dram_tensor("xp", [NB, CI, HP, WP], F16, kind="ExternalInput").ap()
    w_d = nc.dram_tensor("wts", [128, 256], F16, kind="ExternalInput").ap()
    # device-layout output: [n, strip, batch, (ph,co), (t', gi, j)];
    # host un-permutes to NCHW (out row = 64t + 32b + 4t' + 2gi + ph)
    o_d = nc.dram_tensor(
        "out", [NB, NSS, NEB, 128, EB * 512], F16, kind="ExternalOutput"
    ).ap()

    xe_n = CI * HP * WP        # x_pad element strides
    xe_c = HP * WP
    xe_h = WP

    with tile.TileContext(nc) as tc:
        with (
            tc.tile_pool(name="wp", bufs=1) as wpool,
            tc.tile_pool(name="slab", bufs=4) as slabpool,
            tc.tile_pool(name="evac", bufs=4) as evacpool,
            tc.tile_pool(name="ps", bufs=4, space="PSUM") as pspool,
        ):
            wsb = wpool.tile([128, 256], F16)
            # w1 = wsb[:, 0:128] (pass1, all 128 partitions)
            # w2 = wsb[64:128, 128:256] (pass2, kw1 block partitions)
            # (loads emitted in the prologue, after slab0's loads)

            def copies(sv, g0, g1, pool_c1=0):
                # c1:  kw0_s23[g] = kw0_s01[g+1]      (rows 2g+1, 2g+2)
                #      (optionally first `pool_c1` groups on GpSimd)
                # c23: kw1[g,j]   = kw0[g,j+1]        (64-partition copy)
                if pool_c1:
                    nc.gpsimd.tensor_copy(
                        sv[32:64, g0 : g0 + pool_c1, 0:WP],
                        sv[0:32, g0 + 1 : g0 + pool_c1 + 1, 0:WP],
                    )
                nc.vector.tensor_copy(
                    sv[32:64, g0 + pool_c1 : g1, 0:WP],
                    sv[0:32, g0 + pool_c1 + 1 : g1 + 1, 0:WP],
                )
                nc.vector.tensor_copy(
                    sv[64:128, g0:g1, 0 : WP - 1],
                    sv[0:64, g0:g1, 1:WP],
                )

            def load_slab(n, t, eng=None):
                # slab covers the whole strip: groups 0..G-1 (+ halo slot G)
                slab = slabpool.tile([128, GH * WP], F16, tag="slab")
                sf = slab[:]
                for s in range(2):
                    src = bass.AP(
                        x_d.tensor,
                        n * xe_n + (RSTRIP * t + s) * xe_h,
                        [[xe_c, CI], [2 * xe_h, GH], [1, WP]],
                    )
                    (eng or nc.sync).dma_start(sf[16 * s : 16 * s + 16, :], src)
                return slab

            def load_slab_extras(slab, n, t, eng, eng_s23=None):
                # fill s23 / kw1 blocks straight from HBM (no DVE copies);
                # only worthwhile while the DMA queue still has slack
                sf = slab[:]
                base = n * xe_n + RSTRIP * t * xe_h
                for s in (2, 3):     # kw0_s23: x-rows 2g+s-1, g in 0..G-1
                    src = bass.AP(
                        x_d.tensor,
                        base + s * xe_h,
                        [[xe_c, CI], [2 * xe_h, G], [1, WP]],
                    )
                    (eng_s23 or eng).dma_start(
                        sf[16 * s : 16 * s + 16, 0 : G * WP], src
                    )
                sv = sf.rearrange("p (g j) -> p g j", j=WP)
                for s in range(4):   # kw1: same rows, +1 px, 257 cols
                    src = bass.AP(
                        x_d.tensor,
                        base + s * xe_h + 1,
                        [[xe_c, CI], [2 * xe_h, G], [1, WP - 1]],
                    )
                    eng.dma_start(
                        sv[64 + 16 * s : 80 + 16 * s, 0:G, 0 : WP - 1], src
                    )

            def compute(n, t, slab, all_scalar=False, fine_stores=False,
                        dve_evacs=(3, 7), ebs=range(4)):
                sv = slab[:].rearrange("p (g j) -> p g j", j=WP)
                for eb in ebs:
                    evac = evacpool.tile([128, 2048], F16, tag="evac")
                    for pth in range(2):
                        pt = 2 * eb + pth
                        split = fine_stores and pt == 7
                        if split:
                            pss = [pspool.tile([128, 512], F32, tag="ps",
                                               name=f"ps_h{qq}")
                                   for qq in range(2)]
                        else:
                            ps = pspool.tile([128, 1024], F32, tag="ps")
                        for q in range(2):
                            g0 = 2 * (2 * pt + q)
                            dstq = (pss[q][:] if split
                                    else ps[:, q * 512 : (q + 1) * 512])
                            nc.tensor.matmul(
                                dstq,
                                wsb[0:128, 0:128],
                                sv[0:128, g0 : g0 + 2, 0:256],
                                start=True,
                                stop=False,
                            )
                            nc.tensor.matmul(
                                dstq,
                                wsb[64:128, 128:256],
                                sv[64:128, g0 : g0 + 2, 1:257],
                                start=False,
                                stop=True,
                            )
                            if split:
                                # half-evacs: first overlaps the last mms
                                op = (nc.vector.tensor_copy if q
                                      else nc.scalar.copy)
                                op(
                                    evac[:, pth * 1024 + q * 512 :
                                         pth * 1024 + (q + 1) * 512],
                                    pss[q][:],
                                )
                        if fine_stores:
                            if not split:
                                dve = pt in dve_evacs
                                op = (nc.vector.tensor_copy if dve
                                      else nc.scalar.copy)
                                op(evac[:, pth * 1024 : (pth + 1) * 1024],
                                   ps[:])
                            dst = bass.AP(
                                o_d.tensor,
                                (((n * NSS + t) * 4 + eb) * 128 * 2048
                                 + pth * 1024),
                                [[2048, 128], [1, 1024]],
                            )
                            nc.sync.dma_start(
                                dst, evac[:, pth * 1024 : (pth + 1) * 1024]
                            )
                        else:
                            dve = (pt in dve_evacs) and not all_scalar
                            op = (nc.vector.tensor_copy if dve
                                  else nc.scalar.copy)
                            op(evac[:, pth * 1024 : (pth + 1) * 1024], ps[:])
                    if not fine_stores:
                        dst = bass.AP(
                            o_d.tensor,
                            ((n * NSS + t) * 4 + eb) * 128 * 2048,
                            [[2048, 128], [1, 2048]],
                        )
                        nc.sync.dma_start(dst, evac[:])

            # --- software pipeline ---------------------------------------
            # Loads run 2 steps ahead (SP queue, ahead of stores); copies run
            # 1 step ahead and are emitted BEFORE each step's evacs so the
            # DVE never idles on matmul waits while copies are ready.
            steps = [(n, t) for n in range(NB) for t in range(NSS)]

            # prologue: slab 0 in chunks for a fast start (dep tracking is
            # region-precise, so early matmuls run on partial slabs);
            # high_priority pins these ahead of later copies in the scheduler
            slab0 = slabpool.tile([128, GH * WP], F16, tag="slab")
            sv0 = slab0[:].rearrange("p (g j) -> p g j", j=WP)
            CHUNKS = (0, 8, 16, 24, 32)   # pair-group boundaries
            with tc.high_priority():
                # disjoint load ranges (no WAR chains between chunks)
                for c in range(4):
                    ga = 0 if c == 0 else CHUNKS[c] + 1
                    gb = CHUNKS[c + 1] + 1
                    for s in range(2):
                        leng = (nc.sync if (c < 2 and (c > 0 or s == 0))
                                else nc.gpsimd)
                        src = bass.AP(
                            x_d.tensor,
                            (2 * ga + s) * xe_h,
                            [[xe_c, CI], [2 * xe_h, gb - ga], [1, WP]],
                        )
                        leng.dma_start(
                            slab0[:][16 * s : 16 * s + 16,
                                     ga * WP : gb * WP],
                            src,
                        )
                    if c == 0:
                        nc.sync.dma_start(wsb[:], w_d[0:128, :])
                for c in range(4):
                    copies(sv0, CHUNKS[c], CHUNKS[c + 1])
            slabs = {steps[0]: slab0}
            slabs[steps[2]] = load_slab(*steps[2], eng=nc.sync)
            slabs[steps[1]] = load_slab(*steps[1], eng=nc.gpsimd)
            load_slab_extras(slabs[steps[1]], *steps[1], nc.sync,
                             eng_s23=nc.gpsimd)
            for i, (n, t) in enumerate(steps):
                if i + 3 < len(steps):
                    slabs[steps[i + 3]] = load_slab(*steps[i + 3])
                nxt = (slabs[steps[i + 2]][:].rearrange(
                           "p (g j) -> p g j", j=WP)
                       if i + 2 < len(steps) else None)
                last = i == len(steps) - 1
                slab = slabs.pop((n, t))
                kw = dict(all_scalar=(i == 0), fine_stores=last,
                          dve_evacs=(1, 3, 5, 7) if last else (2, 5))
                if nxt is not None:
                    # DVE stream: c1(i+2), evac(i,3), c23(i+2), evac(i,6)
                    nc.vector.tensor_copy(
                        nxt[32:64, 0:G, 0:WP],
                        nxt[0:32, 1 : G + 1, 0:WP],
                    )
                    compute(n, t, slab, ebs=range(2), **kw)
                    nc.vector.tensor_copy(
                        nxt[64:128, 0:G, 0 : WP - 1],
                        nxt[0:64, 0:G, 1:WP],
                    )
                    compute(n, t, slab, ebs=range(2, 4), **kw)
                else:
                    compute(n, t, slab, **kw)

    nc.compile()
    return nc


def _prep_weights(W: np.ndarray) -> np.ndarray:
    # lhsT layouts, stored stacked as [192, 128] then transposed on load:
    #   w1[(d,s,ci), (ph,co)] = W[co, ci, s-ph, d]    (rows 0..127)
    #   w2[(s,ci),   (ph,co)] = W[co, ci, s-ph, 2]    (rows 128..191)
    w = np.zeros((128, 256), dtype=np.float32)
    for s in range(4):
        for ph in range(2):
            kh = s - ph
            if not (0 <= kh <= 2):
                continue
            blk = W[:, :, kh, :]  # [co, ci, kw]
            for d in range(2):
                w[d * 64 + s * 16 : d * 64 + (s + 1) * 16,
                  ph * 64 : (ph + 1) * 64] = blk[:, :, d].T
            w[64 + s * 16 : 64 + (s + 1) * 16,
              128 + ph * 64 : 128 + (ph + 1) * 64] = blk[:, :, 2].T
    return w.astype(np.float16)


def _prep_inputs(x: np.ndarray, W: np.ndarray) -> list[dict]:
    wts = _prep_weights(np.asarray(W, dtype=np.float32))
    xs = np.asarray(x, dtype=np.float32).reshape(NCORES, NB, CI, H, W_SP)
    in_maps = []
    for i in range(NCORES):
        xp = np.zeros((NB, CI, HP, WP), dtype=np.float16)
        xp[:, :, 1 : H + 1, 1 : W_SP + 1] = xs[i]
        in_maps.append({"xp": xp, "wts": wts})
    return in_maps


def kernel(x: np.ndarray, W: np.ndarray) -> np.ndarray:
    assert x.shape == (N_FULL, CI, H, W_SP) and W.shape == (CO, CI, 3, 3)
    # BASS_TRACE without the axon NTFF hook module would crash the run path;
    # disable tracing only when the hook is genuinely unavailable.
    try:
        import antenv.axon_hooks  # noqa: F401
    except Exception:
        import os

        os.environ.setdefault("BASS_NEVER_TRACE", "1")
    if "nc" not in _CACHE:
        _CACHE["nc"] = _build()
    nc = _CACHE["nc"]

    in_maps = _prep_inputs(x, W)
    res = run_bass_kernel_spmd(nc, in_maps, list(range(NCORES)))
    parts = []
    for i in range(NCORES):
        dev = np.asarray(res.results[i]["out"], dtype=np.float32)
        # [n, t, eb, ph, co, h, gi, j] -> [n, co, (t eb h gi ph), j]
        # (out row = 64t + 16eb + 4h + 2gi + ph)
        dev = dev.reshape(NB, NSS, 4, 2, CO, 4, 2, 256)
        dev = dev.transpose(0, 4, 1, 2, 5, 6, 3, 7).reshape(NB, CO, H, W_SP)
        parts.append(dev)
    return np.concatenate(parts, axis=0)
